# revision 15
# baseline (speedup 1.0000x reference)
"""CGCNN (3x CGConv + BatchNorm) on 8 Trainium2 NeuronCores.

Strategy: data-parallel over node ranges (6250 nodes/core); each core owns the
edges targeting its nodes. The edge matmuls z@W decompose into node-level
tables A=h@Wf_t, B=h@Wf_s, C=h@Ws_t, D=h@Ws_s plus per-edge gathers/adds:
  gate_pre = A[tgt]+B[src]+e@Wf_e+bf,  core_pre = C[tgt]+D[src]+e@Ws_e+bs
Tables are fp16; [A|C] is tgt-indexed (core-local), [B|D] src-indexed and
all-gathered across cores each layer. Per-edge work runs edge-major:
dma_gather -> DVE adds -> ACT sigmoid / exp+ln1p -> msg -> one-hot scatter
matmul into per-128-node-block PSUM accumulators. BN stats via AllReduce.

Edges are sorted by (tgt block, src-half, src) and padded per (block, half) to
multiples of 128 with a cross-core max so all 8 cores run one SPMD program.
The src tables are gathered in two halves (rows <32768 / >=32768) because
dma_gather indices are int16.
"""
import sys
import numpy as np

sys.path.insert(0, "/opt/trn_rl_repo")

import concourse.bass as bass  # noqa: E402
import concourse.mybir as mybir  # noqa: E402
import concourse.tile as tile  # noqa: E402
from concourse import bacc  # noqa: E402
from concourse.bass_utils import run_bass_kernel_spmd  # noqa: E402
from concourse.masks import make_identity  # noqa: E402
from concourse.bass import ts  # noqa: E402

N = 50000
E = 800000
OF = 92   # orig_fea
NF = 41   # nbr_fea
H = 64
NCONV = 3
NCORES = 8
NPC = N // NCORES       # 6250
P = 128
SPLIT = 32768
NBLK = (NPC + P - 1) // P  # 49
BN_EPS = 1e-5
GT = 16                  # target tiles per processing group

F16 = mybir.dt.float16
F32 = mybir.dt.float32
I16 = mybir.dt.int16
I32 = mybir.dt.int32

# debug knobs (leave True/3 for production)
CC_ON = True      # use collectives (else: local DMA placeholders, wrong results)
EDGES_ON = True   # run the per-edge message-passing loop
EDGE_STAGE = 3    # 1=gathers only, 2=+compute, 3=+scatter (full)


# ----------------------------------------------------------------------------
# host-side prep
# ----------------------------------------------------------------------------

def _wrap16(ix):
    """int16 gather-index layout: element j at [j%16, j//16], replicated x8."""
    n = len(ix)
    a = np.asarray(ix, np.int16).reshape(n // 16, 16).T.copy()  # [16, n/16]
    return np.tile(a, (8, 1))  # [128, n/16]


def _prep(inputs):
    src = np.asarray(inputs["edge_index"][0]).astype(np.int64)
    tgt = np.asarray(inputs["edge_index"][1]).astype(np.int64)
    edge_attr = np.asarray(inputs["x" if False else "edge_attr"], np.float32)

    percore = []
    counts = np.zeros((NCORES, NBLK, 2), np.int64)
    for c in range(NCORES):
        m = (tgt >= c * NPC) & (tgt < (c + 1) * NPC)
        eids = np.nonzero(m)[0]
        tl = tgt[eids] - c * NPC
        s = src[eids]
        blk = tl // P
        half = (s >= SPLIT).astype(np.int64)
        order = np.lexsort((s, half, blk))
        eids, tl, s, blk, half = (a[order] for a in (eids, tl, s, blk, half))
        percore.append((eids, tl, s, blk, half))
        for k in range(NBLK):
            mk = blk == k
            counts[c, k, 0] = np.sum(mk & (half == 0))
            counts[c, k, 1] = np.sum(mk & (half == 1))

    TL = tuple(int(np.ceil(counts[:, k, 0].max() / P)) for k in range(NBLK))
    TH = tuple(int(np.ceil(counts[:, k, 1].max() / P)) for k in range(NBLK))

    in_maps = []
    for c in range(NCORES):
        eids, tl, s, blk, half = percore[c]
        low_idx, high_idx, proc_eid, proc_tloc = [], [], [], []
        for k in range(NBLK):
            for h, T, acc in ((0, TL[k], low_idx), (1, TH[k], high_idx)):
                mk = (blk == k) & (half == h)
                ss = s[mk]
                tt = tl[mk] - k * P
                ee = eids[mk]
                pad = T * P - len(ss)
                acc.append(np.concatenate([ss - (SPLIT if h else 0),
                                           np.zeros(pad, np.int64)]))
                proc_eid.append(np.concatenate([ee, -np.ones(pad, np.int64)]))
                proc_tloc.append(np.concatenate([tt, 999 * np.ones(pad, np.int64)]))
        low_idx = np.concatenate(low_idx)
        high_idx = np.concatenate(high_idx)
        proc_eid = np.concatenate(proc_eid)
        proc_tloc = np.concatenate(proc_tloc)
        EP = len(proc_eid)

        # static scatter one-hot, edge-partition layout: pgT[e, t*128+n]
        pgT = np.zeros((P, EP), np.float16)
        vs = np.nonzero(proc_tloc < P)[0]
        pgT[vs % P, (vs // P) * P + proc_tloc[vs]] = 1.0

        ac_idx = np.zeros(EP, np.int64)
        blk_seq = np.concatenate([np.full(int((TL[k] + TH[k]) * P), k)
                                  for k in range(NBLK)])
        valid = proc_eid >= 0
        ac_idx[valid] = blk_seq[valid] * P + proc_tloc[valid]

        ea = np.zeros((EP, NF), np.float32)
        ea[valid] = edge_attr[proc_eid[valid]]

        x = np.asarray(inputs["x"], np.float32)
        Wf = np.asarray(inputs["Wf"], np.float32)
        Ws = np.asarray(inputs["Ws"], np.float32)
        bf = np.asarray(inputs["bf"], np.float32)
        bs = np.asarray(inputs["bs"], np.float32)

        Wt = np.stack([np.concatenate([Wf[i][:H], Ws[i][:H]], 1) for i in range(NCONV)])
        Wt_aug = np.concatenate(
            [Wt, np.stack([np.concatenate([bf[i], bs[i]])[None, :] for i in range(NCONV)])],
            axis=1)  # [3, 65, 128]
        Wsrc = np.stack([np.concatenate([Wf[i][H:2 * H], Ws[i][H:2 * H]], 1)
                         for i in range(NCONV)])  # [3, 64, 128]
        We = np.stack([np.concatenate([Wf[i][2 * H:], Ws[i][2 * H:]], 1)
                       for i in range(NCONV)]).astype(np.float16)  # [3, 64, 128]

        in_maps.append({
            "xT": x[c * NPC:(c + 1) * NPC].T.astype(np.float16).copy(),
            "eaT": ea.T.astype(np.float16).copy(),
            "idx_ac": _wrap16(ac_idx),
            "idx_low": _wrap16(low_idx),
            "idx_high": _wrap16(high_idx),
            "tgtloc": proc_tloc.astype(np.float32).reshape(EP // P, P).T.copy(),
            "pgT": pgT,
            "W1": np.asarray(inputs["W1"], np.float16),
            "b1": np.asarray(inputs["b1"], np.float32).reshape(H, 1),
            "W2": np.asarray(inputs["W2"], np.float16),
            "b2": np.asarray(inputs["b2"], np.float32).reshape(H, 1),
            "Wt": np.ascontiguousarray(np.transpose(Wt_aug, (1, 0, 2))).reshape(H + 1, NCONV * 2 * H).astype(np.float16),
            "Wsrc": np.ascontiguousarray(np.transpose(Wsrc, (1, 0, 2))).reshape(H, NCONV * 2 * H).astype(np.float16),
            "We": np.ascontiguousarray(np.transpose(We, (1, 0, 2))).reshape(H, NCONV * 2 * H),
            "gam": np.asarray(inputs["gamma"], np.float32).T.copy(),  # [64, 3]
            "bet": np.asarray(inputs["beta"], np.float32).T.copy(),
        })
    return in_maps, TL, TH


# ----------------------------------------------------------------------------
# device program
# ----------------------------------------------------------------------------

def _groups(TL, TH):
    cumL = np.concatenate([[0], np.cumsum(TL)])
    cumH = np.concatenate([[0], np.cumsum(TH)])
    gs = []
    k0 = 0
    while k0 < NBLK:
        k1 = k0
        nt = 0
        while k1 < NBLK and nt < GT:
            nt += TL[k1] + TH[k1]
            k1 += 1
        gs.append(dict(k0=k0, k1=k1,
                       lt0=int(cumL[k0]), lt1=int(cumL[k1]),
                       ht0=int(cumH[k0]), ht1=int(cumH[k1]),
                       pt0=int(cumL[k0] + cumH[k0]), pt1=int(cumL[k1] + cumH[k1])))
        k0 = k1
    return gs


def _build(TL, TH):
    TLs, THs = sum(TL), sum(TH)
    EP = (TLs + THs) * P
    NL, NH = TLs * P, THs * P
    groups = _groups(TL, TH)
    cumL = np.concatenate([[0], np.cumsum(TL)])
    cumH = np.concatenate([[0], np.cumsum(TH)])

    nc = bacc.Bacc("TRN2", target_bir_lowering=False, debug=False,
                   num_devices=NCORES, num_swdge_queues=4)

    # params
    xT = nc.dram_tensor("xT", [OF, NPC], F16, kind="ExternalInput").ap()
    eaT = nc.dram_tensor("eaT", [NF, EP], F16, kind="ExternalInput").ap()
    idx_ac = nc.dram_tensor("idx_ac", [P, EP // 16], I16, kind="ExternalInput").ap()
    idx_low = nc.dram_tensor("idx_low", [P, NL // 16], I16, kind="ExternalInput").ap()
    idx_high = nc.dram_tensor("idx_high", [P, NH // 16], I16, kind="ExternalInput").ap()
    tgtloc = nc.dram_tensor("tgtloc", [P, EP // P], F32, kind="ExternalInput").ap()
    pgT_d = nc.dram_tensor("pgT", [P, EP], F16, kind="ExternalInput").ap()
    W1 = nc.dram_tensor("W1", [OF, H], F16, kind="ExternalInput").ap()
    b1 = nc.dram_tensor("b1", [H, 1], F32, kind="ExternalInput").ap()
    W2 = nc.dram_tensor("W2", [NF, H], F16, kind="ExternalInput").ap()
    b2 = nc.dram_tensor("b2", [H, 1], F32, kind="ExternalInput").ap()
    Wt = nc.dram_tensor("Wt", [H + 1, NCONV * 2 * H], F16, kind="ExternalInput").ap()
    Wsrc = nc.dram_tensor("Wsrc", [H, NCONV * 2 * H], F16, kind="ExternalInput").ap()
    We = nc.dram_tensor("We", [H, NCONV * 2 * H], F16, kind="ExternalInput").ap()
    gam = nc.dram_tensor("gam", [H, NCONV], F32, kind="ExternalInput").ap()
    bet = nc.dram_tensor("bet", [H, NCONV], F32, kind="ExternalInput").ap()
    out = nc.dram_tensor("out", [H, NPC], F32, kind="ExternalOutput").ap()

    # internals
    eT16 = nc.dram_tensor("eT16", [H, EP], F16)
    ac_tab = [nc.dram_tensor(f"ac_tab{i}", [NPC, 2 * H], F16) for i in range(NCONV)]
    bd_loc = [nc.dram_tensor(f"bd_loc{i}", [NPC, 2 * H], F16) for i in range(NCONV)]
    bd_full = [nc.dram_tensor(f"bd_full{i}", [N, 2 * H], F16, addr_space="Shared")
               for i in range(NCONV)]
    cc_in = [nc.dram_tensor(f"cc_in{i}", [H, 2], F32) for i in range(NCONV)]
    cc_out = [nc.dram_tensor(f"cc_out{i}", [H, 2], F32, addr_space="Shared")
              for i in range(NCONV)]

    with tile.TileContext(nc) as tc:
        with (
            tc.tile_pool(name="const", bufs=1) as cp,
            tc.tile_pool(name="hp", bufs=1) as hp,
            tc.tile_pool(name="st", bufs=2) as st,
            tc.tile_pool(name="sm", bufs=2) as sm,
            tc.tile_pool(name="mmp", bufs=2, space="PSUM") as mmp,
            tc.tile_pool(name="efp", bufs=2, space="PSUM") as efp,
            tc.tile_pool(name="agp", bufs=2, space="PSUM") as agp,
            tc.tile_pool(name="trp", bufs=1, space="PSUM") as trp,
        ):
            # ---- constants
            ident = cp.tile([P, P], F32, name="ident")
            make_identity(nc, ident[:])
            iota_i = cp.tile([P, P], I32, name="iota_i")
            nc.gpsimd.iota(iota_i[:], pattern=[[1, P]], base=0, channel_multiplier=0)
            iota_f = cp.tile([P, P], F32, name="iota_f")
            nc.vector.tensor_copy(iota_f[:], iota_i[:])

            w1s = cp.tile([OF, H], F16, name="w1s")
            nc.sync.dma_start(w1s[:], W1[:, :])
            b1s = cp.tile([H, 1], F32, name="b1s")
            nc.sync.dma_start(b1s[:], b1[:, :])
            w2s = cp.tile([NF, H], F16, name="w2s")
            nc.sync.dma_start(w2s[:], W2[:, :])
            b2s = cp.tile([H, 1], F32, name="b2s")
            nc.sync.dma_start(b2s[:], b2[:, :])
            wts = cp.tile([H + 1, NCONV * 2 * H], F16, name="wts")
            nc.sync.dma_start(wts[:], Wt[:, :])
            wss = cp.tile([H, NCONV * 2 * H], F16, name="wss")
            nc.sync.dma_start(wss[:], Wsrc[:, :])
            wes = cp.tile([H, NCONV * 2 * H], F16, name="wes")
            nc.sync.dma_start(wes[:], We[:, :])
            gams = cp.tile([H, NCONV], F32, name="gams")
            nc.sync.dma_start(gams[:], gam[:, :])
            bets = cp.tile([H, NCONV], F32, name="bets")
            nc.sync.dma_start(bets[:], bet[:, :])

            iac = cp.tile([P, EP // 16], I16, name="iac")
            nc.sync.dma_start(iac[:], idx_ac[:, :])
            ilo = cp.tile([P, NL // 16], I16, name="ilo")
            nc.sync.dma_start(ilo[:], idx_low[:, :])
            ihi = cp.tile([P, NH // 16], I16, name="ihi")
            nc.sync.dma_start(ihi[:], idx_high[:, :])
            tls = cp.tile([P, EP // P], F32, name="tls")
            nc.sync.dma_start(tls[:], tgtloc[:, :])

            # ---- embedding: h0^T = (x @ W1 + b1)^T, feature-major + ones row
            hcur = hp.tile([H + 1, NPC], F16, name="hcur0", tag="hcur", bufs=2)
            CH = 512
            for a in range(0, NPC, CH):
                n = min(CH, NPC - a)
                xc = st.tile([OF, CH], F16, name=f"xc{a}", tag="xc")
                nc.sync.dma_start(xc[:, :n], xT[:, a:a + n])
                pe = mmp.tile([H, CH], F32, name=f"pe{a}", tag="mmp")
                nc.tensor.matmul(pe[:, :n], lhsT=w1s[:], rhs=xc[:, :n],
                                 start=True, stop=True)
                nc.vector.tensor_scalar(out=hcur[0:H, a:a + n], in0=pe[:, :n],
                                        scalar1=b1s[:], scalar2=None,
                                        op0=mybir.AluOpType.add)
            nc.vector.memset(hcur[H:H + 1, :], 1.0)

            # ---- e~^T = (edge_attr @ W2 + b2)^T  (fp16, DRAM scratch)
            for a in range(0, EP, CH):
                n = min(CH, EP - a)
                ec = st.tile([NF, CH], F16, name=f"ec{a}", tag="ec")
                nc.sync.dma_start(ec[:, :n], eaT[:, a:a + n])
                pe = mmp.tile([H, CH], F32, name=f"pee{a}", tag="mmp")
                nc.tensor.matmul(pe[:, :n], lhsT=w2s[:], rhs=ec[:, :n],
                                 start=True, stop=True)
                e16 = st.tile([H, CH], F16, name=f"e16{a}", tag="e16")
                nc.vector.tensor_scalar(out=e16[:, :n], in0=pe[:, :n],
                                        scalar1=b2s[:], scalar2=None,
                                        op0=mybir.AluOpType.add)
                nc.sync.dma_start(eT16[:, a:a + n], e16[:, :n])

            # ---- layers
            for li in range(NCONV):
                wt_l = wts[:, ts(li, 2 * H)]
                ws_l = wss[:, ts(li, 2 * H)]
                we_l = wes[:, ts(li, 2 * H)]

                # tables
                for j in range(NBLK):
                    a = j * P
                    nj = min(P, NPC - a)
                    tp = mmp.tile([nj, 2 * H], F32, name=f"tp{li}_{j}", tag="mmp")
                    nc.tensor.matmul(tp[:], lhsT=hcur[0:H + 1, a:a + nj],
                                     rhs=wt_l, start=True, stop=True)
                    t16 = st.tile([nj, 2 * H], F16, name=f"t16{li}_{j}", tag="t16")
                    nc.vector.tensor_copy(t16[:], tp[:])
                    nc.sync.dma_start(ac_tab[li][a:a + nj, :], t16[:])

                    tp2 = mmp.tile([nj, 2 * H], F32, name=f"tq{li}_{j}", tag="mmp")
                    nc.tensor.matmul(tp2[:], lhsT=hcur[0:H, a:a + nj],
                                     rhs=ws_l, start=True, stop=True)
                    t16b = st.tile([nj, 2 * H], F16, name=f"t16b{li}_{j}", tag="t16")
                    nc.vector.tensor_copy(t16b[:], tp2[:])
                    nc.sync.dma_start(bd_loc[li][a:a + nj, :], t16b[:])

                if CC_ON:
                    nc.gpsimd.collective_compute(
                        "AllGather", mybir.AluOpType.bypass,
                        replica_groups=[list(range(NCORES))],
                        ins=[bd_loc[li].ap().opt()],
                        outs=[bd_full[li].ap().opt()],
                    )
                else:
                    nc.sync.dma_start(bd_full[li][0:NPC, :], bd_loc[li][:, :])

                h2 = hp.tile([H, NPC], F16, name=f"h2_{li}", tag="h2", bufs=1)

                if not EDGES_ON or EDGE_STAGE < 3:
                    nc.vector.tensor_copy(h2[:], hcur[0:H, :])

                # edge pass
                for gi, g in enumerate(groups if EDGES_ON else []):
                    k0, k1 = g["k0"], g["k1"]
                    nlt, nht = g["lt1"] - g["lt0"], g["ht1"] - g["ht0"]
                    ngt = g["pt1"] - g["pt0"]
                    pt0 = g["pt0"]

                    acst = st.tile([P, ngt, P], F16, name=f"ac{li}_{gi}", tag="acst")
                    nc.gpsimd.dma_gather(
                        out_ap=acst[:], in_ap=ac_tab[li][:, :],
                        idxs_ap=iac[:, pt0 * 8:(pt0 + ngt) * 8],
                        num_idxs=ngt * P, num_idxs_reg=ngt * P, elem_size=2 * H,
                        single_packet=False, queue_num=gi % 4)
                    if nlt:
                        lost = st.tile([P, nlt, P], F16, name=f"lo{li}_{gi}", tag="lost")
                        nc.gpsimd.dma_gather(
                            out_ap=lost[:], in_ap=bd_full[li][0:SPLIT, :],
                            idxs_ap=ilo[:, g["lt0"] * 8:g["lt1"] * 8],
                            num_idxs=nlt * P, num_idxs_reg=nlt * P, elem_size=2 * H,
                            single_packet=False, queue_num=(gi + 1) % 4)
                    if nht:
                        hist = st.tile([P, nht, P], F16, name=f"hi{li}_{gi}", tag="hist")
                        nc.gpsimd.dma_gather(
                            out_ap=hist[:], in_ap=bd_full[li][SPLIT:N, :],
                            idxs_ap=ihi[:, g["ht0"] * 8:g["ht1"] * 8],
                            num_idxs=nht * P, num_idxs_reg=nht * P, elem_size=2 * H,
                            single_packet=False, queue_num=(gi + 2) % 4)

                    ets = st.tile([H, ngt * P], F16, name=f"et{li}_{gi}", tag="ets")
                    nc.sync.dma_start(ets[:], eT16[:, pt0 * P:(pt0 + ngt) * P])
                    ptg = st.tile([P, ngt, P], F16, name=f"pg{li}_{gi}", tag="ptg")
                    nc.sync.dma_start(ptg[:], pgT_d[:, pt0 * P:(pt0 + ngt) * P])

                    if EDGE_STAGE < 2:
                        continue
                    ust = st.tile([P, ngt, P], F16, name=f"u{li}_{gi}", tag="ust")
                    for k in range(k0, k1):
                        tlk, thk = TL[k], TH[k]
                        pk = int(cumL[k] + cumH[k]) - pt0
                        if tlk:
                            la = int(cumL[k]) - g["lt0"]
                            nc.vector.tensor_tensor(
                                out=ust[:, pk:pk + tlk, :],
                                in0=acst[:, pk:pk + tlk, :],
                                in1=lost[:, la:la + tlk, :],
                                op=mybir.AluOpType.add)
                        if thk:
                            ha = int(cumH[k]) - g["ht0"]
                            nc.vector.tensor_tensor(
                                out=ust[:, pk + tlk:pk + tlk + thk, :],
                                in0=acst[:, pk + tlk:pk + tlk + thk, :],
                                in1=hist[:, ha:ha + thk, :],
                                op=mybir.AluOpType.add)

                    for t in range(ngt):
                        ef = efp.tile([P, 2 * H], F32, name=f"ef{li}_{gi}_{t}", tag="efp")
                        nc.tensor.matmul(ef[:], lhsT=ets[:, t * P:(t + 1) * P],
                                         rhs=we_l, start=True, stop=True)
                        nc.vector.tensor_tensor(out=ust[:, t, :], in0=ust[:, t, :],
                                                in1=ef[:], op=mybir.AluOpType.add)

                    gst = sm.tile([P, ngt, H], F16, name=f"g{li}_{gi}", tag="gst")
                    nc.scalar.activation(gst[:], ust[:, :, 0:H],
                                         mybir.ActivationFunctionType.Sigmoid)
                    tex = sm.tile([P, ngt, H], F16, name=f"tx{li}_{gi}", tag="tex")
                    nc.scalar.activation(tex[:], ust[:, :, H:2 * H],
                                         mybir.ActivationFunctionType.Exp)
                    nc.scalar.activation(tex[:], tex[:],
                                         mybir.ActivationFunctionType.Ln, bias=1.0)
                    msg = gst
                    nc.vector.tensor_tensor(out=msg[:], in0=gst[:], in1=tex[:],
                                            op=mybir.AluOpType.mult)

                    if EDGE_STAGE < 3:
                        continue
                    for k in range(k0, k1):
                        ntk = TL[k] + TH[k]
                        if ntk == 0:
                            continue
                        pk = int(cumL[k] + cumH[k]) - pt0
                        ag = agp.tile([P, H], F32, name=f"ag{li}_{k}", tag="agp")
                        for j in range(ntk):
                            r = pk + j
                            nc.tensor.matmul(ag[:], lhsT=ptg[:, r, :], rhs=msg[:, r, :],
                                             start=(j == 0), stop=(j == ntk - 1))
                        agsb = sm.tile([P, H], F32, name=f"ags{li}_{k}", tag="agsb")
                        nc.vector.tensor_copy(agsb[:], ag[:])
                        agt = trp.tile([H, P], F32, name=f"agt{li}_{k}", tag="trp")
                        nc.tensor.transpose(agt[:], agsb[:], ident[:])
                        a = k * P
                        sz = min(P, NPC - a)
                        nc.vector.tensor_tensor(out=h2[:, a:a + sz],
                                                in0=hcur[0:H, a:a + sz],
                                                in1=agt[:, 0:sz],
                                                op=mybir.AluOpType.add)

                # ---- BN stats
                nchunk = (NPC + CH - 1) // CH
                s1r = sm.tile([H, nchunk], F32, name=f"s1r{li}", tag="s1r")
                s2r = sm.tile([H, nchunk], F32, name=f"s2r{li}", tag="s2r")
                for ci, a in enumerate(range(0, NPC, CH)):
                    n = min(CH, NPC - a)
                    nc.vector.tensor_reduce(s1r[:, ci:ci + 1], h2[:, a:a + n],
                                            axis=mybir.AxisListType.X,
                                            op=mybir.AluOpType.add)
                    sq = sm.tile([H, CH], F16, name=f"sq{li}_{ci}", tag="sq")
                    nc.vector.tensor_tensor(out=sq[:, :n], in0=h2[:, a:a + n],
                                            in1=h2[:, a:a + n],
                                            op=mybir.AluOpType.mult)
                    nc.vector.tensor_reduce(s2r[:, ci:ci + 1], sq[:, :n],
                                            axis=mybir.AxisListType.X,
                                            op=mybir.AluOpType.add)
                stat = sm.tile([H, 2], F32, name=f"stat{li}", tag="stat")
                nc.vector.tensor_reduce(stat[:, 0:1], s1r[:],
                                        axis=mybir.AxisListType.X,
                                        op=mybir.AluOpType.add)
                nc.vector.tensor_reduce(stat[:, 1:2], s2r[:],
                                        axis=mybir.AxisListType.X,
                                        op=mybir.AluOpType.add)
                nc.sync.dma_start(cc_in[li][:, :], stat[:])
                if CC_ON:
                    nc.gpsimd.collective_compute(
                        "AllReduce", mybir.AluOpType.add,
                        replica_groups=[list(range(NCORES))],
                        ins=[cc_in[li].ap().opt()],
                        outs=[cc_out[li].ap().opt()],
                    )
                else:
                    nc.sync.dma_start(cc_out[li][:, :], cc_in[li][:, :])
                statg = sm.tile([H, 2], F32, name=f"statg{li}", tag="statg")
                nc.sync.dma_start(statg[:], cc_out[li][:, :])

                mean = sm.tile([H, 1], F32, name=f"mean{li}", tag="mean")
                nc.vector.tensor_scalar(out=mean[:], in0=statg[:, 0:1],
                                        scalar1=1.0 / N, scalar2=None,
                                        op0=mybir.AluOpType.mult)
                var = sm.tile([H, 1], F32, name=f"var{li}", tag="var")
                # var = s2/N - mean^2
                mm2 = sm.tile([H, 1], F32, name=f"mm2{li}", tag="mm2")
                nc.vector.tensor_tensor(out=mm2[:], in0=mean[:], in1=mean[:],
                                        op=mybir.AluOpType.mult)
                nc.vector.tensor_scalar(out=var[:], in0=statg[:, 1:2],
                                        scalar1=1.0 / N, scalar2=None,
                                        op0=mybir.AluOpType.mult)
                nc.vector.tensor_tensor(out=var[:], in0=var[:], in1=mm2[:],
                                        op=mybir.AluOpType.subtract)
                nc.vector.tensor_scalar(out=var[:], in0=var[:],
                                        scalar1=BN_EPS, scalar2=None,
                                        op0=mybir.AluOpType.add)
                std = sm.tile([H, 1], F32, name=f"std{li}", tag="std")
                nc.scalar.activation(std[:], var[:],
                                     mybir.ActivationFunctionType.Sqrt)
                inv = sm.tile([H, 1], F32, name=f"inv{li}", tag="inv")
                nc.vector.reciprocal(inv[:], std[:])
                scl = sm.tile([H, 1], F32, name=f"scl{li}", tag="scl")
                nc.vector.tensor_tensor(out=scl[:], in0=inv[:], in1=gams[:, li:li + 1],
                                        op=mybir.AluOpType.mult)
                shf = sm.tile([H, 1], F32, name=f"shf{li}", tag="shf")
                nc.vector.tensor_tensor(out=shf[:], in0=mean[:], in1=scl[:],
                                        op=mybir.AluOpType.mult)
                nc.vector.tensor_tensor(out=shf[:], in0=bets[:, li:li + 1], in1=shf[:],
                                        op=mybir.AluOpType.subtract)

                if li < NCONV - 1:
                    hnew = hp.tile([H + 1, NPC], F16, name=f"hc{li + 1}", tag="hcur",
                                   bufs=2)
                    nc.scalar.activation(hnew[0:H, :], h2[:],
                                         mybir.ActivationFunctionType.Relu,
                                         bias=shf[:], scale=scl[:])
                    nc.vector.memset(hnew[H:H + 1, :], 1.0)
                    hcur = hnew
                else:
                    hfin = hp.tile([H, NPC], F32, name="hfin", tag="hcur", bufs=2)
                    nc.vector.tensor_scalar(out=hfin[:], in0=h2[:],
                                            scalar1=scl[:], scalar2=shf[:],
                                            op0=mybir.AluOpType.mult,
                                            op1=mybir.AluOpType.add)
                    nc.sync.dma_start(out[:, :], hfin[:])

    nc.compile()
    return nc


_CACHE = {}


def kernel(**inputs):
    in_maps, TL, TH = _prep(inputs)
    key = (TL, TH)
    if key not in _CACHE:
        _CACHE[key] = _build(TL, TH)
    nc = _CACHE[key]
    res = run_bass_kernel_spmd(nc, in_maps, core_ids=list(range(NCORES)))
    outs = [res.results[c]["out"] for c in range(NCORES)]
    return np.concatenate([o.T for o in outs], axis=0).astype(np.float32)


if __name__ == "__main__":
    pass

